# revision 24
# baseline (speedup 1.0000x reference)
"""Trainium2 Bass kernel for Euler-integrated Kuramoto dynamics.

    dtheta_i/dt = omega_i + sum_j K[i,j] * sin(theta_j - theta_i)

Strategy (8 NeuronCores, SPMD):
  sin(theta_j - theta_i) = sin(theta_j)cos(theta_i) - cos(theta_j)sin(theta_i)
so the per-step coupling reduction is two matvecs against K:
  coupling = cos(theta) * (K @ sin(theta)) - sin(theta) * (K @ cos(theta))

K is sharded row-wise: core c owns rows [512c, 512c+512). The shard is
staged as lhsT tiles (fp16) resident in SBUF for all 50 steps; the matvec
runs K-stationary with a (128, 2) moving sin/cos operand (the LDWEIGHTS/
MATMUL pairs pipeline at ~28ns, ~3.7us/step). Each step every core updates
its own 512 phases, then the sin/cos of the updated shard (fp16, 2 KB) is
exchanged so the next step's matvec has the full sin/cos vector.

All SBUF layouts pack each 512-shard as (128 partitions, 4 cols) with
local element l = 4*p + a - partition-major - so every SBUF<->DRAM bounce
transfer is a contiguous 16B-per-partition run (an l = 128*a + p mapping
makes the bounce DMAs scatter 2-byte packets which swamp all 16 DMA
engines; completion semaphores then lag 9-20us).

Exchange paths:
  cc (default): DRAM-bounce AllGather via collective_compute. Mesh AG of
    2KB has a ~4.6us floor; with the two bounce DMAs the exchange costs
    ~10us/step of the ~21us step.
  rdma (KUR_RDMA=1, experimental — measured WORSE here): direct SBUF->SBUF
    remote_dma_broadcast to the 7 peers with XOR-relative addressing (slot
    lam of core d holds the shard of core d^lam; k-tile order XOR-permuted
    host-side per core; SC double-buffered by step parity; rs-semaphore
    waits injected post-scheduling because the per-core scheduling sim
    cannot see remote increments). On this runtime the SWDGE remote sends
    take ~50us/round (vs ~1us modeled) and instructions keep at most 2
    wait conditions (extra injected waits are silently dropped), so this
    path is both slow and, with >2 pending slots, unsafe. Kept for
    reference.

Scalar-engine Sin is only valid on [-pi, pi]; phases drift outside, so
inputs are range-reduced with f = u - round(u) in turns and
sin = Sin(2*pi*f) (cos via the +0.25-turn offset).
"""

import numpy as np

N = 4096
M = 8  # cores
S = N // M  # 512 phases per core
NT = N // 128  # 32 contraction k-tiles
IT = S // 128  # 4 output i-tiles per core
import os as _os

N_STEPS = int(_os.environ.get("KUR_STEPS", "50"))
RDMA = bool(int(_os.environ.get("KUR_RDMA", "0")))
MODSC = bool(int(_os.environ.get("KUR_MODSC", "0")))  # DVE mod fails walrus ISA check
SHARED = bool(int(_os.environ.get("KUR_SHARED", "1")))
SPLITIN = bool(int(_os.environ.get("KUR_SPLITIN", "1")))
NO_CC = bool(int(_os.environ.get("KUR_NO_CC", "0")))
NO_MM = bool(int(_os.environ.get("KUR_NO_MM", "0")))
NO_DMA = bool(int(_os.environ.get("KUR_NO_DMA", "0")))
EPISPLIT = bool(int(_os.environ.get("KUR_EPISPLIT", "1")))
FP8 = bool(int(_os.environ.get("KUR_FP8", "0")))  # slower AND 100x worse rel err (1e-2)
KSCALE = 32.0  # pre-scale for fp8 K to clear the e4m3 denormal range
DT = 0.01
PI = 3.141592653589793

TRACE = False
LAST_RESULTS = None

_compiled_nc = None


def _build(n_steps=None, rdma=None, no_cc=NO_CC, no_mm=NO_MM, no_dma=NO_DMA):
    import concourse.bass as bass  # noqa: F401
    import concourse.tile as tile
    from concourse import bacc, mybir
    from concourse.tile_rust import add_dep_helper

    if n_steps is None:
        n_steps = N_STEPS
    if rdma is None:
        rdma = RDMA

    f32 = mybir.dt.float32
    f16 = mybir.dt.float16
    AF = mybir.ActivationFunctionType
    OP = mybir.AluOpType

    nc = bacc.Bacc(
        "TRN2",
        target_bir_lowering=False,
        debug=False,
        enable_asserts=False,
        num_devices=M,
    )
    f8 = mybir.dt.float8e4
    kdt = f8 if FP8 else f16
    kt = nc.dram_tensor("kt", [N, S], kdt, kind="ExternalInput").ap()
    ph = nc.dram_tensor("ph", [N], f32, kind="ExternalInput").ap()
    th0 = nc.dram_tensor("th0", [S], f32, kind="ExternalInput").ap()
    om = nc.dram_tensor("om", [S], f32, kind="ExternalInput").ap()  # dt*omega shard
    th_out = nc.dram_tensor("th_out", [S], f32, kind="ExternalOutput").ap()

    if rdma:
        # per-slot remote semaphores (slot 0 = own, unused) + send-drain sem
        rs = [nc.alloc_semaphore(f"rdma_rs{l}") if l else None for l in range(M)]
        ls = nc.alloc_semaphore("rdma_ls")
    # (instruction, [(sem, threshold), ...]) wait conditions injected AFTER
    # Tile scheduling — the remote increments are invisible to the per-core
    # scheduling sim and would deadlock it.
    patches = []

    with tile.TileContext(nc) as tc:
        with (
            tc.tile_pool(name="pers", bufs=1) as pers,
            tc.tile_pool(name="psum", bufs=2, space="PSUM") as psum_pool,
            tc.tile_pool(name="work", bufs=2) as work,
            tc.tile_pool(name="dram", bufs=2, space="DRAM") as dram,
        ):
            KT = pers.tile([128, NT * S], kdt)  # k-tile t at cols [t*512,(t+1)*512)
            # sin/cos of the full 4096 vector: col 2t = sin_t, col 2t+1 = cos_t.
            # rdma: double-buffered by step parity; slot lam = cols [8lam, 8lam+8)
            SCA = pers.tile([128, 2 * NT], f16, tag="sca")
            SCB = pers.tile([128, 2 * NT], f16, tag="scb", name="SCB") if rdma else None
            SCo = None if rdma else pers.tile([128, 2 * IT], f16, tag="sco", name="SCo")
            T = pers.tile([128, IT], f32)  # own theta shard
            OM = pers.tile([128, IT], f32)  # dt*omega shard
            NPI = pers.tile([128, 1], f32)  # -pi activation bias
            nc.gpsimd.memset(NPI[:], -PI)

            # --- preamble: K resident load + initial sin/cos of full phases ---
            for t in range(NT):
                nc.sync.dma_start(KT[:, t * S : (t + 1) * S], kt[t * 128 : (t + 1) * 128, :])
            nc.sync.dma_start(T[:], th0.rearrange("(p a) -> p a", p=128))
            nc.sync.dma_start(OM[:], om.rearrange("(p a) -> p a", p=128))
            T0f = work.tile([128, NT], f32, tag="t0f")
            nc.sync.dma_start(
                T0f.rearrange("p (c a) -> p c a", c=M, a=IT),
                ph.rearrange("(c p a) -> p c a", c=M, p=128, a=IT),
            )

            INV2PI = 1.0 / (2.0 * PI)
            # (u + BIG) - BIG == round-to-nearest-integer(u) in fp32; the 1.5x
            # keeps u + BIG inside [2^23, 2^24) (ulp exactly 1) for negative u too
            BIG = 1.5 * 2.0**23

            def emit_sincos(dst_sin, dst_cos, src, shape_cols, tag):
                # Scalar-engine Sin is only valid on [-pi, pi]: range-reduce.
                acts = []
                if MODSC:
                    # m = mod(theta + off, 2pi) in [0, 2pi); sin(theta) =
                    # Sin(m - pi) since theta = m - off (mod 2pi), off odd*pi.
                    # cos via the extra +pi/2 inside the mod. One DVE op each.
                    for dst, off, nm in ((dst_sin, 9.0 * PI, "s"), (dst_cos, 9.5 * PI, "c")):
                        mm_ = work.tile([128, shape_cols], f32, tag=f"m{nm}{tag}")
                        nc.vector.tensor_scalar(mm_[:], src, off, 2.0 * PI, OP.add, OP.mod)
                        acts.append(
                            nc.scalar.activation(dst, mm_[:], AF.Sin, bias=NPI[:])
                        )
                    return acts
                # f = u - round(u) in turns-of-2pi, then Sin(f * 2pi).
                for dst, quarter, nm in ((dst_sin, 0.0, "s"), (dst_cos, 0.25, "c")):
                    u = work.tile([128, shape_cols], f32, tag=f"u{nm}{tag}")
                    w = work.tile([128, shape_cols], f32, tag=f"w{nm}{tag}")
                    f = work.tile([128, shape_cols], f32, tag=f"f{nm}{tag}")
                    nc.vector.tensor_scalar(u[:], src, INV2PI, quarter, OP.mult, OP.add)
                    nc.vector.tensor_scalar(w[:], u[:], BIG, BIG, OP.add, OP.subtract)
                    nc.vector.tensor_tensor(f[:], u[:], w[:], OP.subtract)
                    acts.append(nc.scalar.activation(dst, f[:], AF.Sin, scale=2.0 * PI))
                return acts

            emit_sincos(SCA[:, 0::2], SCA[:, 1::2], T0f[:], NT, "f")
            if not rdma:
                emit_sincos(SCo[:, 0::2], SCo[:, 1::2], T[:], IT, "o")

            for s in range(n_steps):
                if rdma:
                    buf = SCA if s % 2 == 0 else SCB
                    nxt = SCB if s % 2 == 0 else SCA
                    own_sin = buf[:, 0 : 2 * IT : 2]
                    own_cos = buf[:, 1 : 2 * IT : 2]
                else:
                    buf = SCA
                    own_sin = SCo[:, 0::2]
                    own_cos = SCo[:, 1::2]

                ps = psum_pool.tile([128, 2 * IT], f32)
                first_mm = None
                for it in range(IT if not no_mm else 1):
                    base = it * 128
                    for t in range(NT if not no_mm else 1):
                        mm = nc.tensor.matmul(
                            ps[:, 2 * it : 2 * it + 2],
                            lhsT=KT[:, t * S + base : t * S + base + 128],
                            rhs=buf[:, 2 * t : 2 * t + 2],  # {sin_t, cos_t}
                            start=(t == 0),
                            stop=(t == (NT - 1 if not no_mm else 0)),
                        )
                        if t == 0:
                            if it == 0:
                                first_mm = mm
                                if rdma and s > 0:
                                    # the scheduler's single-core sim cannot
                                    # see remote increments, so the actual
                                    # rs-waits are injected post-scheduling;
                                    # here just record the gate instruction.
                                    patches.append(
                                        (mm.ins, [(rs[l], 2 * s) for l in range(1, M)])
                                    )
                            else:
                                # keep it0-t0 first on the PE queue so its
                                # injected waits gate every matmul of the step
                                add_dep_helper(
                                    mm.ins, first_mm.ins, sync=False,
                                    reason="step's first matmul carries the rdma waits",
                                )
                # coupling = cos_own * (K@sin) - sin_own * (K@cos);  T += dt*coupling + dt*omega
                a = work.tile([128, IT], f32, tag="a")
                b = work.tile([128, IT], f32, tag="b")
                d = work.tile([128, IT], f32, tag="d")
                tom = work.tile([128, IT], f32, tag="tom")
                kinv = DT / KSCALE if FP8 else DT
                nc.vector.tensor_tensor(tom[:], T[:], OM[:], OP.add)
                if EPISPLIT:
                    # per-i-tile: i-tile it's update starts as soon as its own
                    # 32 matmuls finish instead of after the whole matvec
                    ob = buf if rdma else SCo
                    for it in range(IT):
                        sl1 = slice(it, it + 1)
                        nc.vector.tensor_tensor(
                            a[:, sl1], ob[:, 2 * it + 1 : 2 * it + 2], ps[:, 2 * it : 2 * it + 1], OP.mult
                        )
                        nc.vector.tensor_tensor(
                            b[:, sl1], ob[:, 2 * it : 2 * it + 1], ps[:, 2 * it + 1 : 2 * it + 2], OP.mult
                        )
                        nc.vector.tensor_tensor(d[:, sl1], a[:, sl1], b[:, sl1], OP.subtract)
                        nc.vector.scalar_tensor_tensor(
                            T[:, sl1], d[:, sl1], kinv, tom[:, sl1], OP.mult, OP.add
                        )
                else:
                    nc.vector.tensor_tensor(a[:], own_cos, ps[:, 0::2], OP.mult)
                    nc.vector.tensor_tensor(b[:], own_sin, ps[:, 1::2], OP.mult)
                    nc.vector.tensor_tensor(d[:], a[:], b[:], OP.subtract)
                    nc.vector.scalar_tensor_tensor(T[:], d[:], kinv, tom[:], OP.mult, OP.add)

                if s < n_steps - 1:
                    if rdma:
                        # write own sincos straight into slot 0 of the next
                        # parity buffer, then fire 7 direct SBUF->SBUF sends
                        acts = emit_sincos(
                            nxt[:, 0 : 2 * IT : 2], nxt[:, 1 : 2 * IT : 2], T[:], IT, "o"
                        )
                        if s >= 2:
                            # slot-0 region of nxt was last read by the sends
                            # triggered at the end of step s-2; make sure they
                            # drained before overwriting the source.
                            for a_ in acts:
                                patches.append((a_.ins, [(ls, 112 * (s - 1))]))
                        for l in range(1, M):
                            rdests = [None] * M
                            rdests[l] = (0, l)  # XOR-relative: dest = self ^ l
                            nc.gpsimd.remote_dma_broadcast(
                                nxt[:, 8 * l : 8 * l + 8],
                                nxt[:, 0:8],
                                rs[l],
                                ls,
                                rdests=rdests,
                            )
                        nc.gpsimd.trigger_dma(count=None)
                    else:
                        emit_sincos(SCo[:, 0::2], SCo[:, 1::2], T[:], IT, "o")
                        cin = dram.tile([2 * S], f16, tag="cin")
                        cout = dram.tile(
                            [2 * S * M], f16, tag="cout",
                            addr_space="Shared" if SHARED else "Local",
                        )
                        if not no_dma:
                            # cin element p*8 + a*2 + h  <-  SCo[p, 2a+h]
                            nc.sync.dma_start(
                                cin.rearrange("(p a h) -> p a h", p=128, a=IT, h=2),
                                SCo.rearrange("p (a h) -> p a h", h=2),
                            )
                        if not no_cc:
                            nc.gpsimd.collective_compute(
                                "AllGather",
                                OP.bypass,
                                replica_groups=[list(range(M))],
                                ins=[cin.opt()],
                                outs=[cout.opt()],
                            )
                        if not no_dma:
                            # SC[p, 8c+2a+h]  <-  cout element c*1024 + p*8 + a*2 + h
                            if SPLITIN:
                                # halves on two queues land in parallel and
                                # matmuls for low-rank k-tiles start earlier
                                H = M // 2
                                nc.sync.dma_start(
                                    SCA[:, : M * IT].rearrange(
                                        "p (c a h) -> p c a h", c=H, a=IT, h=2
                                    ),
                                    cout[: S * M].rearrange(
                                        "(c p a h) -> p c a h", c=H, p=128, a=IT, h=2
                                    ),
                                )
                                nc.scalar.dma_start(
                                    SCA[:, M * IT :].rearrange(
                                        "p (c a h) -> p c a h", c=H, a=IT, h=2
                                    ),
                                    cout[S * M :].rearrange(
                                        "(c p a h) -> p c a h", c=H, p=128, a=IT, h=2
                                    ),
                                )
                            else:
                                nc.sync.dma_start(
                                    SCA.rearrange("p (c a h) -> p c a h", c=M, a=IT, h=2),
                                    cout.rearrange("(c p a h) -> p c a h", c=M, p=128, a=IT, h=2),
                                )

            nc.sync.dma_start(th_out.rearrange("(p a) -> p a", p=128), T[:])

    for inst, conds in patches:
        si = inst.sync_info
        on_wait = list(si.on_wait) if si is not None else []
        on_update = list(si.on_update) if si is not None else []
        for sem, val in conds:
            on_wait.append(
                mybir.SyncWait(
                    sync_type="semaphore",
                    id=sem.num,
                    wait_mode="sem-ge-imm",
                    wait_value=val,
                    ant_name=sem.name,
                )
            )
        inst.sync_info = mybir.SyncInfo(on_wait=on_wait, on_update=on_update)

    nc.compile()
    return nc


def _get_nc():
    global _compiled_nc
    if _compiled_nc is None:
        _compiled_nc = _build()
    return _compiled_nc


def _cast_kt(kt_c):
    if FP8:
        from concourse import mybir

        return np.ascontiguousarray(kt_c * KSCALE).astype(
            mybir.dt.np(mybir.dt.float8e4)
        )
    return np.ascontiguousarray(kt_c).astype(np.float16)


def kernel(phases, K, omegas):
    global LAST_RESULTS
    from concourse import bass_utils

    phases = np.ascontiguousarray(np.asarray(phases, dtype=np.float32))
    K = np.asarray(K, dtype=np.float32)
    omegas = np.asarray(omegas, dtype=np.float32)

    nc = _get_nc()
    in_maps = []
    for c in range(M):
        sl = slice(c * S, (c + 1) * S)
        # kt[tau*128 + p, it*128 + q] = K[own + 4q + it, 512*gb + 4p + aa]
        # with tau = 4*lam + aa and gb = global block held in slot lam:
        # rdma: gb = c ^ lam (XOR-relative remote writes); cc: gb = lam.
        A = K[sl, :]  # [i_loc, j]
        B = A.reshape(128, IT, M, 128, IT)  # [q, it, gb, p, aa]
        if RDMA:
            perm = [c ^ l for l in range(M)]
            B = B[:, :, perm, :, :]
            ph_in = phases.reshape(M, S)[perm].reshape(-1)
        else:
            ph_in = phases
        kt_c = B.transpose(2, 4, 3, 1, 0).reshape(N, S)
        in_maps.append(
            {
                "kt": _cast_kt(kt_c),
                "ph": np.ascontiguousarray(ph_in),
                "th0": np.ascontiguousarray(phases[sl]),
                "om": np.ascontiguousarray(DT * omegas[sl]).astype(np.float32),
            }
        )
    res = bass_utils.run_bass_kernel_spmd(
        nc, in_maps, core_ids=list(range(M)), trace=TRACE
    )
    LAST_RESULTS = res
    out = np.concatenate([res.results[c]["th_out"] for c in range(M)])
    return out.astype(np.float32)


# revision 25
# speedup vs baseline: 1.0825x; 1.0825x over previous
"""Trainium2 Bass kernel for Euler-integrated Kuramoto dynamics.

    dtheta_i/dt = omega_i + sum_j K[i,j] * sin(theta_j - theta_i)

Strategy (8 NeuronCores, SPMD):
  sin(theta_j - theta_i) = sin(theta_j)cos(theta_i) - cos(theta_j)sin(theta_i)
so the per-step coupling reduction is two matvecs against K:
  coupling = cos(theta) * (K @ sin(theta)) - sin(theta) * (K @ cos(theta))

K is sharded row-wise: core c owns rows [512c, 512c+512). The shard is
staged as lhsT tiles (fp16) resident in SBUF for all 50 steps; the matvec
runs K-stationary with a (128, 2) moving sin/cos operand (the LDWEIGHTS/
MATMUL pairs pipeline at ~28ns, ~3.7us/step). Each step every core updates
its own 512 phases, then the sin/cos of the updated shard (fp16, 2 KB) is
exchanged so the next step's matvec has the full sin/cos vector.

All SBUF layouts pack each 512-shard as (128 partitions, 4 cols) with
local element l = 4*p + a - partition-major - so every SBUF<->DRAM bounce
transfer is a contiguous 16B-per-partition run (an l = 128*a + p mapping
makes the bounce DMAs scatter 2-byte packets which swamp all 16 DMA
engines; completion semaphores then lag 9-20us).

Exchange paths:
  cc (default): DRAM-bounce AllGather via collective_compute. Mesh AG of
    2KB has a ~4.6us floor; with the two bounce DMAs the exchange costs
    ~10us/step of the ~21us step.
  rdma (KUR_RDMA=1, experimental — measured WORSE here): direct SBUF->SBUF
    remote_dma_broadcast to the 7 peers with XOR-relative addressing (slot
    lam of core d holds the shard of core d^lam; k-tile order XOR-permuted
    host-side per core; SC double-buffered by step parity; rs-semaphore
    waits injected post-scheduling because the per-core scheduling sim
    cannot see remote increments). On this runtime the SWDGE remote sends
    take ~50us/round (vs ~1us modeled) and instructions keep at most 2
    wait conditions (extra injected waits are silently dropped), so this
    path is both slow and, with >2 pending slots, unsafe. Kept for
    reference.

Scalar-engine Sin is only valid on [-pi, pi]; phases drift outside, so
inputs are range-reduced with f = u - round(u) in turns and
sin = Sin(2*pi*f) (cos via the +0.25-turn offset).
"""

import numpy as np

N = 4096
M = 8  # cores
S = N // M  # 512 phases per core
NT = N // 128  # 32 contraction k-tiles
IT = S // 128  # 4 output i-tiles per core
import os as _os

N_STEPS = int(_os.environ.get("KUR_STEPS", "50"))
RDMA = bool(int(_os.environ.get("KUR_RDMA", "0")))
MODSC = bool(int(_os.environ.get("KUR_MODSC", "0")))  # DVE mod fails walrus ISA check
SHARED = bool(int(_os.environ.get("KUR_SHARED", "1")))
SPLITIN = bool(int(_os.environ.get("KUR_SPLITIN", "1")))
NO_CC = bool(int(_os.environ.get("KUR_NO_CC", "0")))
NO_MM = bool(int(_os.environ.get("KUR_NO_MM", "0")))
NO_DMA = bool(int(_os.environ.get("KUR_NO_DMA", "0")))
EPISPLIT = bool(int(_os.environ.get("KUR_EPISPLIT", "0")))  # measured slower: small-op overhead > overlap gain
FP8 = bool(int(_os.environ.get("KUR_FP8", "0")))  # slower AND 100x worse rel err (1e-2)
KSCALE = 32.0  # pre-scale for fp8 K to clear the e4m3 denormal range
DT = 0.01
PI = 3.141592653589793

TRACE = False
LAST_RESULTS = None

_compiled_nc = None


def _build(n_steps=None, rdma=None, no_cc=NO_CC, no_mm=NO_MM, no_dma=NO_DMA):
    import concourse.bass as bass  # noqa: F401
    import concourse.tile as tile
    from concourse import bacc, mybir
    from concourse.tile_rust import add_dep_helper

    if n_steps is None:
        n_steps = N_STEPS
    if rdma is None:
        rdma = RDMA

    f32 = mybir.dt.float32
    f16 = mybir.dt.float16
    AF = mybir.ActivationFunctionType
    OP = mybir.AluOpType

    nc = bacc.Bacc(
        "TRN2",
        target_bir_lowering=False,
        debug=False,
        enable_asserts=False,
        num_devices=M,
    )
    f8 = mybir.dt.float8e4
    kdt = f8 if FP8 else f16
    kt = nc.dram_tensor("kt", [N, S], kdt, kind="ExternalInput").ap()
    ph = nc.dram_tensor("ph", [N], f32, kind="ExternalInput").ap()
    th0 = nc.dram_tensor("th0", [S], f32, kind="ExternalInput").ap()
    om = nc.dram_tensor("om", [S], f32, kind="ExternalInput").ap()  # dt*omega shard
    th_out = nc.dram_tensor("th_out", [S], f32, kind="ExternalOutput").ap()

    if rdma:
        # per-slot remote semaphores (slot 0 = own, unused) + send-drain sem
        rs = [nc.alloc_semaphore(f"rdma_rs{l}") if l else None for l in range(M)]
        ls = nc.alloc_semaphore("rdma_ls")
    # (instruction, [(sem, threshold), ...]) wait conditions injected AFTER
    # Tile scheduling — the remote increments are invisible to the per-core
    # scheduling sim and would deadlock it.
    patches = []

    with tile.TileContext(nc) as tc:
        with (
            tc.tile_pool(name="pers", bufs=1) as pers,
            tc.tile_pool(name="psum", bufs=2, space="PSUM") as psum_pool,
            tc.tile_pool(name="work", bufs=2) as work,
            tc.tile_pool(name="dram", bufs=2, space="DRAM") as dram,
        ):
            KT = pers.tile([128, NT * S], kdt)  # k-tile t at cols [t*512,(t+1)*512)
            # sin/cos of the full 4096 vector: col 2t = sin_t, col 2t+1 = cos_t.
            # rdma: double-buffered by step parity; slot lam = cols [8lam, 8lam+8)
            SCA = pers.tile([128, 2 * NT], f16, tag="sca")
            SCB = pers.tile([128, 2 * NT], f16, tag="scb", name="SCB") if rdma else None
            SCo = None if rdma else pers.tile([128, 2 * IT], f16, tag="sco", name="SCo")
            T = pers.tile([128, IT], f32)  # own theta shard
            OM = pers.tile([128, IT], f32)  # dt*omega shard
            NPI = pers.tile([128, 1], f32)  # -pi activation bias
            nc.gpsimd.memset(NPI[:], -PI)

            # --- preamble: K resident load + initial sin/cos of full phases ---
            for t in range(NT):
                nc.sync.dma_start(KT[:, t * S : (t + 1) * S], kt[t * 128 : (t + 1) * 128, :])
            nc.sync.dma_start(T[:], th0.rearrange("(p a) -> p a", p=128))
            nc.sync.dma_start(OM[:], om.rearrange("(p a) -> p a", p=128))
            T0f = work.tile([128, NT], f32, tag="t0f")
            nc.sync.dma_start(
                T0f.rearrange("p (c a) -> p c a", c=M, a=IT),
                ph.rearrange("(c p a) -> p c a", c=M, p=128, a=IT),
            )

            INV2PI = 1.0 / (2.0 * PI)
            # (u + BIG) - BIG == round-to-nearest-integer(u) in fp32; the 1.5x
            # keeps u + BIG inside [2^23, 2^24) (ulp exactly 1) for negative u too
            BIG = 1.5 * 2.0**23

            def emit_sincos(dst_sin, dst_cos, src, shape_cols, tag):
                # Scalar-engine Sin is only valid on [-pi, pi]: range-reduce.
                acts = []
                if MODSC:
                    # m = mod(theta + off, 2pi) in [0, 2pi); sin(theta) =
                    # Sin(m - pi) since theta = m - off (mod 2pi), off odd*pi.
                    # cos via the extra +pi/2 inside the mod. One DVE op each.
                    for dst, off, nm in ((dst_sin, 9.0 * PI, "s"), (dst_cos, 9.5 * PI, "c")):
                        mm_ = work.tile([128, shape_cols], f32, tag=f"m{nm}{tag}")
                        nc.vector.tensor_scalar(mm_[:], src, off, 2.0 * PI, OP.add, OP.mod)
                        acts.append(
                            nc.scalar.activation(dst, mm_[:], AF.Sin, bias=NPI[:])
                        )
                    return acts
                # f = u - round(u) in turns-of-2pi, then Sin(f * 2pi).
                for dst, quarter, nm in ((dst_sin, 0.0, "s"), (dst_cos, 0.25, "c")):
                    u = work.tile([128, shape_cols], f32, tag=f"u{nm}{tag}")
                    w = work.tile([128, shape_cols], f32, tag=f"w{nm}{tag}")
                    f = work.tile([128, shape_cols], f32, tag=f"f{nm}{tag}")
                    nc.vector.tensor_scalar(u[:], src, INV2PI, quarter, OP.mult, OP.add)
                    nc.vector.tensor_scalar(w[:], u[:], BIG, BIG, OP.add, OP.subtract)
                    nc.vector.tensor_tensor(f[:], u[:], w[:], OP.subtract)
                    acts.append(nc.scalar.activation(dst, f[:], AF.Sin, scale=2.0 * PI))
                return acts

            emit_sincos(SCA[:, 0::2], SCA[:, 1::2], T0f[:], NT, "f")
            if not rdma:
                emit_sincos(SCo[:, 0::2], SCo[:, 1::2], T[:], IT, "o")

            for s in range(n_steps):
                if rdma:
                    buf = SCA if s % 2 == 0 else SCB
                    nxt = SCB if s % 2 == 0 else SCA
                    own_sin = buf[:, 0 : 2 * IT : 2]
                    own_cos = buf[:, 1 : 2 * IT : 2]
                else:
                    buf = SCA
                    own_sin = SCo[:, 0::2]
                    own_cos = SCo[:, 1::2]

                ps = psum_pool.tile([128, 2 * IT], f32)
                first_mm = None
                for it in range(IT if not no_mm else 1):
                    base = it * 128
                    for t in range(NT if not no_mm else 1):
                        mm = nc.tensor.matmul(
                            ps[:, 2 * it : 2 * it + 2],
                            lhsT=KT[:, t * S + base : t * S + base + 128],
                            rhs=buf[:, 2 * t : 2 * t + 2],  # {sin_t, cos_t}
                            start=(t == 0),
                            stop=(t == (NT - 1 if not no_mm else 0)),
                        )
                        if t == 0:
                            if it == 0:
                                first_mm = mm
                                if rdma and s > 0:
                                    # the scheduler's single-core sim cannot
                                    # see remote increments, so the actual
                                    # rs-waits are injected post-scheduling;
                                    # here just record the gate instruction.
                                    patches.append(
                                        (mm.ins, [(rs[l], 2 * s) for l in range(1, M)])
                                    )
                            else:
                                # keep it0-t0 first on the PE queue so its
                                # injected waits gate every matmul of the step
                                add_dep_helper(
                                    mm.ins, first_mm.ins, sync=False,
                                    reason="step's first matmul carries the rdma waits",
                                )
                # coupling = cos_own * (K@sin) - sin_own * (K@cos);  T += dt*coupling + dt*omega
                a = work.tile([128, IT], f32, tag="a")
                b = work.tile([128, IT], f32, tag="b")
                d = work.tile([128, IT], f32, tag="d")
                tom = work.tile([128, IT], f32, tag="tom")
                kinv = DT / KSCALE if FP8 else DT
                nc.vector.tensor_tensor(tom[:], T[:], OM[:], OP.add)
                if EPISPLIT:
                    # per-i-tile: i-tile it's update starts as soon as its own
                    # 32 matmuls finish instead of after the whole matvec
                    ob = buf if rdma else SCo
                    for it in range(IT):
                        sl1 = slice(it, it + 1)
                        nc.vector.tensor_tensor(
                            a[:, sl1], ob[:, 2 * it + 1 : 2 * it + 2], ps[:, 2 * it : 2 * it + 1], OP.mult
                        )
                        nc.vector.tensor_tensor(
                            b[:, sl1], ob[:, 2 * it : 2 * it + 1], ps[:, 2 * it + 1 : 2 * it + 2], OP.mult
                        )
                        nc.vector.tensor_tensor(d[:, sl1], a[:, sl1], b[:, sl1], OP.subtract)
                        nc.vector.scalar_tensor_tensor(
                            T[:, sl1], d[:, sl1], kinv, tom[:, sl1], OP.mult, OP.add
                        )
                else:
                    nc.vector.tensor_tensor(a[:], own_cos, ps[:, 0::2], OP.mult)
                    nc.vector.tensor_tensor(b[:], own_sin, ps[:, 1::2], OP.mult)
                    nc.vector.tensor_tensor(d[:], a[:], b[:], OP.subtract)
                    nc.vector.scalar_tensor_tensor(T[:], d[:], kinv, tom[:], OP.mult, OP.add)

                if s < n_steps - 1:
                    if rdma:
                        # write own sincos straight into slot 0 of the next
                        # parity buffer, then fire 7 direct SBUF->SBUF sends
                        acts = emit_sincos(
                            nxt[:, 0 : 2 * IT : 2], nxt[:, 1 : 2 * IT : 2], T[:], IT, "o"
                        )
                        if s >= 2:
                            # slot-0 region of nxt was last read by the sends
                            # triggered at the end of step s-2; make sure they
                            # drained before overwriting the source.
                            for a_ in acts:
                                patches.append((a_.ins, [(ls, 112 * (s - 1))]))
                        for l in range(1, M):
                            rdests = [None] * M
                            rdests[l] = (0, l)  # XOR-relative: dest = self ^ l
                            nc.gpsimd.remote_dma_broadcast(
                                nxt[:, 8 * l : 8 * l + 8],
                                nxt[:, 0:8],
                                rs[l],
                                ls,
                                rdests=rdests,
                            )
                        nc.gpsimd.trigger_dma(count=None)
                    else:
                        emit_sincos(SCo[:, 0::2], SCo[:, 1::2], T[:], IT, "o")
                        cin = dram.tile([2 * S], f16, tag="cin")
                        cout = dram.tile(
                            [2 * S * M], f16, tag="cout",
                            addr_space="Shared" if SHARED else "Local",
                        )
                        if not no_dma:
                            # cin element p*8 + a*2 + h  <-  SCo[p, 2a+h]
                            nc.sync.dma_start(
                                cin.rearrange("(p a h) -> p a h", p=128, a=IT, h=2),
                                SCo.rearrange("p (a h) -> p a h", h=2),
                            )
                        if not no_cc:
                            nc.gpsimd.collective_compute(
                                "AllGather",
                                OP.bypass,
                                replica_groups=[list(range(M))],
                                ins=[cin.opt()],
                                outs=[cout.opt()],
                            )
                        if not no_dma:
                            # SC[p, 8c+2a+h]  <-  cout element c*1024 + p*8 + a*2 + h
                            if SPLITIN:
                                # halves on two queues land in parallel and
                                # matmuls for low-rank k-tiles start earlier
                                H = M // 2
                                nc.sync.dma_start(
                                    SCA[:, : M * IT].rearrange(
                                        "p (c a h) -> p c a h", c=H, a=IT, h=2
                                    ),
                                    cout[: S * M].rearrange(
                                        "(c p a h) -> p c a h", c=H, p=128, a=IT, h=2
                                    ),
                                )
                                nc.scalar.dma_start(
                                    SCA[:, M * IT :].rearrange(
                                        "p (c a h) -> p c a h", c=H, a=IT, h=2
                                    ),
                                    cout[S * M :].rearrange(
                                        "(c p a h) -> p c a h", c=H, p=128, a=IT, h=2
                                    ),
                                )
                            else:
                                nc.sync.dma_start(
                                    SCA.rearrange("p (c a h) -> p c a h", c=M, a=IT, h=2),
                                    cout.rearrange("(c p a h) -> p c a h", c=M, p=128, a=IT, h=2),
                                )

            nc.sync.dma_start(th_out.rearrange("(p a) -> p a", p=128), T[:])

    for inst, conds in patches:
        si = inst.sync_info
        on_wait = list(si.on_wait) if si is not None else []
        on_update = list(si.on_update) if si is not None else []
        for sem, val in conds:
            on_wait.append(
                mybir.SyncWait(
                    sync_type="semaphore",
                    id=sem.num,
                    wait_mode="sem-ge-imm",
                    wait_value=val,
                    ant_name=sem.name,
                )
            )
        inst.sync_info = mybir.SyncInfo(on_wait=on_wait, on_update=on_update)

    nc.compile()
    return nc


def _get_nc():
    global _compiled_nc
    if _compiled_nc is None:
        _compiled_nc = _build()
    return _compiled_nc


def _cast_kt(kt_c):
    if FP8:
        from concourse import mybir

        return np.ascontiguousarray(kt_c * KSCALE).astype(
            mybir.dt.np(mybir.dt.float8e4)
        )
    return np.ascontiguousarray(kt_c).astype(np.float16)


def kernel(phases, K, omegas):
    global LAST_RESULTS
    from concourse import bass_utils

    phases = np.ascontiguousarray(np.asarray(phases, dtype=np.float32))
    K = np.asarray(K, dtype=np.float32)
    omegas = np.asarray(omegas, dtype=np.float32)

    nc = _get_nc()
    in_maps = []
    for c in range(M):
        sl = slice(c * S, (c + 1) * S)
        # kt[tau*128 + p, it*128 + q] = K[own + 4q + it, 512*gb + 4p + aa]
        # with tau = 4*lam + aa and gb = global block held in slot lam:
        # rdma: gb = c ^ lam (XOR-relative remote writes); cc: gb = lam.
        A = K[sl, :]  # [i_loc, j]
        B = A.reshape(128, IT, M, 128, IT)  # [q, it, gb, p, aa]
        if RDMA:
            perm = [c ^ l for l in range(M)]
            B = B[:, :, perm, :, :]
            ph_in = phases.reshape(M, S)[perm].reshape(-1)
        else:
            ph_in = phases
        kt_c = B.transpose(2, 4, 3, 1, 0).reshape(N, S)
        in_maps.append(
            {
                "kt": _cast_kt(kt_c),
                "ph": np.ascontiguousarray(ph_in),
                "th0": np.ascontiguousarray(phases[sl]),
                "om": np.ascontiguousarray(DT * omegas[sl]).astype(np.float32),
            }
        )
    res = bass_utils.run_bass_kernel_spmd(
        nc, in_maps, core_ids=list(range(M)), trace=TRACE
    )
    LAST_RESULTS = res
    out = np.concatenate([res.results[c]["th_out"] for c in range(M)])
    return out.astype(np.float32)
